# revision 24
# baseline (speedup 1.0000x reference)
"""Trainium2 Bass kernel for nn_BatteryRNNCell — data-parallel over 8 NeuronCores.

Strategy
--------
Pure data parallel: batch (4M rows) split into 8 x 500K-row shards, one per
core. Each core processes its shard in T=4 tiles of [128 partitions x W=980]
rows (the last tile overlaps the previous one by 1760 rows; overlap rows are
recomputed with identical values, which avoids padding).

Layout: the host transposes `states` to column-major [8, N] before upload and
transposes XNew back after download, so every on-chip access is contiguous
(interleaved row-major state columns cost ~2x on DVE and ~10x on GPSIMD).
State columns are loaded as merged pair tiles (VSNP=[Vsn|Vsp], QB=[qnB|qpB],
QS=[qnS|qpS]) via 2-segment DMAs. Tb passes through via a direct DRAM->DRAM
copy.

Math restructuring (fp32 everywhere except the MLP matmuls):
  asinh(Jn/(2*Jn0)) = ln(2500*i + sqrt(6.25e6*i^2 + 400*un)) - 0.5*ln(400*un)
      with un = xn1000*(1000-xn1000), xn1000 = qnS/qMax  (no division by Jn0)
  sqrt(z) = exp(0.5*ln(z))   (keeps ScalarE inside the natural_log_exp table
      set; the kernel uses only two ACT table sets -> 2 table loads total)
  ln((1-x)/x) = ln(1000-x1000) - ln(x1000)
The 1->8->4->1 tanh MLP runs on the TensorEngine in fp16 (weights fp16,
activations fp16, PSUM accumulation fp32; ~2e-4 max abs error on the MLP
output) via block-replication matmuls; layer 2 packs two 16-row chunks per
PSUM tile so the tanh element count is 12/row instead of 16/row.

GPSIMD is not used at all: its strided ops measured ~19 cyc/elem and its
shared-SBUF-port lock stalls concurrent 2-source VectorE ops for the whole
GPSIMD instruction.
"""

import sys

if "/opt/trn_rl_repo" not in sys.path:
    sys.path.insert(0, "/opt/trn_rl_repo")

from contextlib import ExitStack

import numpy as np

import concourse.bass as bass  # noqa: F401
import concourse.mybir as mybir
import concourse.tile as tile
from concourse import bacc
from concourse.bass import _add_dep_helper
from concourse.bass_utils import run_bass_kernel_spmd

F32 = mybir.dt.float32
F16 = mybir.dt.float16
AF = mybir.ActivationFunctionType
OP = mybir.AluOpType

# ---- physics constants (from the reference model) ----
R_ = 8.3144621
F_ = 96487.0
VOL = 2.2e-05
VOLS = 0.1 * VOL
VOLB = VOL - VOLS
TD = 7.0e6
U0P = 4.03
U0N = 0.01

CB = np.float32(R_ / F_)                 # R/F
CNB = np.float32(1.0 / (VOLB * TD))
CNS = np.float32(1.0 / (VOLS * TD))
C1B = np.float32(1.0 - 1.0 / (VOLB * TD))
C1S = np.float32(1.0 - 1.0 / (VOLS * TD))
C89 = np.float32(1.0 - 1.0 / 90.0)
C45 = np.float32(2.0 / 90.0)             # (R/F/ALPHA)/TSN / (R/F)

NCORES = 8
P = 128
B_FULL = 4_000_000
NPC = B_FULL // NCORES  # 500_000

W = 980
T = 4


def _starts(npc, w, t):
    tile_rows = P * w
    s = [i * tile_rows for i in range(t - 1)]
    s.append(npc - tile_rows)
    assert s[-1] >= (s[-2] if t > 1 else 0)
    return s


def build_nc(npc=NPC, w=W, t_tiles=T):
    starts = _starts(npc, w, t_tiles)
    h = w // 2
    assert h <= 512

    nc = bacc.Bacc(
        "TRN2",
        target_bir_lowering=False,
        debug=False,
        enable_asserts=False,
    )
    # register the 1000.0 activation-bias constant (same pattern as Bass init)
    _k1000 = nc.alloc_sbuf_tensor("const-float32-1000.0", [128, 1], F32)
    nc.gpsimd.memset(_k1000.ap(), 1000.0)
    nc.const_aps.aps[(F32, 1000.0)] = _k1000.ap()
    nc.all_engine_barrier()

    d_inputs = nc.dram_tensor("inputs", [npc, 1], F32, kind="ExternalInput")
    d_states = nc.dram_tensor("statesT", [8, npc], F32, kind="ExternalInput")
    d_qmax = nc.dram_tensor("qMax", [npc], F32, kind="ExternalInput")
    d_ro = nc.dram_tensor("Ro", [npc], F32, kind="ExternalInput")
    d_l1 = nc.dram_tensor("L1s", [128, 1024], F16, kind="ExternalInput")
    d_l2 = nc.dram_tensor("L2p", [128, 256], F16, kind="ExternalInput")
    d_l3 = nc.dram_tensor("L3p", [128, 512], F16, kind="ExternalInput")
    d_b0 = nc.dram_tensor("b0pat", [128, 1], F32, kind="ExternalInput")
    d_b2 = nc.dram_tensor("b2pat", [128, 1], F32, kind="ExternalInput")
    d_wn = nc.dram_tensor("negwn", [128, 1], F32, kind="ExternalInput")
    d_cb = nc.dram_tensor("cbias", [128, 1], F32, kind="ExternalInput")

    d_v = nc.dram_tensor("V", [npc, 1], F32, kind="ExternalOutput")
    d_x = nc.dram_tensor("XNewT", [8, npc], F32, kind="ExternalOutput")

    st_ap = d_states.ap()
    xn_ap = d_x.ap()
    in_flat = d_inputs.ap().rearrange("r c -> (r c)")
    v_flat = d_v.ap().rearrange("r c -> (r c)")
    qm_ap = d_qmax.ap()
    ro_ap = d_ro.ap()

    def vec_slice(flat, s):
        return flat[s : s + P * w].rearrange("(p w) -> p w", w=w)

    def col_slice(c, s, ap=None):
        ap = st_ap if ap is None else ap
        return ap[c, s : s + P * w].rearrange("(p w) -> p w", w=w)

    def pair_slice(ap, c0, step, s):
        # [2, P*w] rows (c0, c0+step) -> 3-D AP matching a [P, 2w] tile
        # viewed as [P, 2, w] ([colA | colB] halves)
        return ap[c0 : c0 + step + 1 : step, s : s + P * w].rearrange(
            "c (p w) -> p c w", w=w
        )

    def pair_tile(t):
        return t[:].rearrange("p (c w) -> p c w", c=2)

    with tile.TileContext(nc) as tc:
        with ExitStack() as ctx:
            cpool = ctx.enter_context(tc.tile_pool(name="const", bufs=1))
            keep = ctx.enter_context(tc.tile_pool(name="keep", bufs=1))

            l1t = cpool.tile([128, 1024], F16, tag="l1")
            nc.sync.dma_start(l1t[:], d_l1.ap())
            l2t = cpool.tile([128, 256], F16, tag="l2")
            nc.sync.dma_start(l2t[:], d_l2.ap())
            l3t = cpool.tile([128, 512], F16, tag="l3")
            nc.sync.dma_start(l3t[:], d_l3.ap())
            b0t = cpool.tile([128, 1], F32, tag="b0")
            nc.sync.dma_start(b0t[:], d_b0.ap())
            b2t = cpool.tile([128, 1], F32, tag="b2")
            nc.sync.dma_start(b2t[:], d_b2.ap())
            wnt = cpool.tile([128, 1], F32, tag="wn")
            nc.sync.dma_start(wnt[:], d_wn.ap())
            cbt = cpool.tile([128, 1], F32, tag="cb")
            nc.sync.dma_start(cbt[:], d_cb.ap())

            # Tb passes through unchanged: one DRAM->DRAM copy
            nc.sync.dma_start(xn_ap[0:1, :], st_ap[0:1, :])

            with ExitStack() as actx:
                inp = actx.enter_context(tc.tile_pool(name="inp", bufs=2))
                big = actx.enter_context(tc.tile_pool(name="big", bufs=1))
                small = actx.enter_context(tc.tile_pool(name="small", bufs=1))
                keep = actx.enter_context(tc.tile_pool(name="keep", bufs=2))
                hpool = actx.enter_context(tc.tile_pool(name="hpool", bufs=1))
                opool = actx.enter_context(tc.tile_pool(name="opool", bufs=2))
                ppool = actx.enter_context(
                    tc.tile_pool(name="ppool", bufs=3, space="PSUM")
                )
                vpool = actx.enter_context(
                    tc.tile_pool(name="vpool", bufs=1, space="PSUM")
                )

                def _half(hh):
                    return slice(512 * hh, 512 * hh + h)

                last_act = [None]

                def act(*a, **kw):
                    # serialize ScalarE in emission order: the scheduler
                    # otherwise interleaves table sets (each flip ~2.7us)
                    r = nc.scalar.activation(*a, **kw)
                    if last_act[0] is not None:
                        _add_dep_helper(
                            r.ins, last_act[0].ins, sync=False, reason="ACT order"
                        )
                    last_act[0] = r
                    return r

                def _half(hh):
                    return slice(512 * hh, 512 * hh + h)

                def emit_mlp(xp16, s1, s):
                    vml = vpool.tile([P, 1024], F32, tag="vml", name="vml")
                    h1s = []
                    for ch in range(8):
                        psa = ppool.tile([P, 1024], F32, tag="ps", name="psa")
                        for hh in range(2):
                            nc.tensor.matmul(
                                psa[:, _half(hh)],
                                l1t[:, ch * 128 : (ch + 1) * 128],
                                xp16[:, hh * h : (hh + 1) * h],
                                start=True,
                                stop=True,
                            )
                        h1 = hpool.tile(
                            [P, 1024], F16, tag=f"h1_{ch}", name=f"h1_{ch}"
                        )
                        act(h1[:], psa[:], AF.Tanh, bias=b0t[:])
                        h1s.append(h1)
                    h2s = []
                    for pc in range(4):
                        psb = ppool.tile([P, 1024], F32, tag="ps", name="psb")
                        for hh in range(2):
                            for e in range(2):
                                nc.tensor.matmul(
                                    psb[:, _half(hh)],
                                    l2t[:, e * 128 : (e + 1) * 128],
                                    h1s[2 * pc + e][:, _half(hh)],
                                    start=(e == 0),
                                    stop=(e == 1),
                                )
                        h2 = hpool.tile(
                            [P, 1024], F16, tag=f"h2_{pc}", name=f"h2_{pc}"
                        )
                        act(h2[:], psb[:], AF.Tanh, bias=b2t[:])
                        h2s.append(h2)
                    for hh in range(2):
                        for pc in range(4):
                            nc.tensor.matmul(
                                vml[:, _half(hh)],
                                l3t[:, pc * 128 : (pc + 1) * 128],
                                h2s[pc][:, _half(hh)],
                                start=(pc == 0),
                                stop=(pc == 3),
                            )
                    vout = opool.tile([P, w], F32, tag="vo2", name="vout")
                    vml3 = vml[:].rearrange("p (b k) -> p b k", k=512)[:, :, :h]
                    s13 = s1[:].rearrange("p (b k) -> p b k", k=h)
                    vo3 = vout[:].rearrange("p (b k) -> p b k", k=h)
                    nc.vector.scalar_tensor_tensor(
                        vo3, vml3, 0.0, s13, OP.add, OP.add
                    )
                    nc.sync.dma_start(vec_slice(v_flat, s), vout[:])

                prev = None  # (xp16, s1, start) of the previous tile
                for ti in range(t_tiles):
                    s = starts[ti]

                    xp16 = keep.tile([P, w], F16, tag="xp", name="xp16")
                    s1 = keep.tile([P, w], F32, tag="s1", name="s1")

                    vin = inp.tile([P, w], F32, tag="vin")
                    nc.sync.dma_start(vin[:], vec_slice(in_flat, s))
                    qq = inp.tile([P, w], F32, tag="qq", bufs=1)
                    nc.sync.dma_start(qq[:], vec_slice(qm_ap, s))
                    rr = inp.tile([P, w], F32, tag="rr", bufs=1)
                    nc.sync.dma_start(rr[:], vec_slice(ro_ap, s))
                    tb = inp.tile([P, w], F32, tag="tb", bufs=1)
                    nc.sync.dma_start(tb[:], col_slice(0, s))
                    vo = inp.tile([P, w], F32, tag="vo")
                    nc.sync.dma_start(vo[:], col_slice(1, s))
                    vsnp = inp.tile([P, 2 * w], F32, tag="vsnp")
                    nc.sync.dma_start(pair_tile(vsnp), pair_slice(st_ap, 2, 1, s))
                    qb = inp.tile([P, 2 * w], F32, tag="qb")
                    nc.sync.dma_start(pair_tile(qb), pair_slice(st_ap, 4, 2, s))
                    qs = inp.tile([P, 2 * w], F32, tag="qs")
                    nc.sync.dma_start(pair_tile(qs), pair_slice(st_ap, 5, 2, s))

                    # scratch
                    xo = big.tile([P, 2 * w], F32, tag="xo")    # x1000|sqm|as2
                    mm = big.tile([P, 2 * w], F32, tag="mm")    # -un|lun|lr1
                    am = big.tile([P, 2 * w], F32, tag="am")    # a1|nm|lnum|lr2
                    lam = big.tile([P, 2 * w], F32, tag="lam")  # lam|x2
                    tnp = big.tile([P, 2 * w], F32, tag="tnp")  # tn|tp , an|ap
                    rq = small.tile([P, w], F32, tag="rq")
                    isq = small.tile([P, w], F32, tag="isq")
                    tbb = small.tile([P, w], F32, tag="tbb")
                    tc2 = big.tile([P, 2 * w], F32, tag="tc2")
                    se1 = small.tile([P, w], F32, tag="se1")
                    se2 = small.tile([P, w], F32, tag="se2")
                    slt = small.tile([P, w], F32, tag="slt")
                    sd2 = small.tile([P, w], F32, tag="sd2")

                    nc.vector.reciprocal_approx_fast(rq[:], qq[:])
                    act(isq[:], vin[:], AF.Square)
                    act(tbb[:], tb[:], AF.Copy, scale=float(CB))
                    act(tc2[:], qs[:], AF.Copy, scale=float(CNS))
                    # x1000 = qS/qMax
                    nc.vector.tensor_tensor(xo[:, :w], qs[:, :w], rq[:], OP.mult)
                    nc.vector.tensor_tensor(xo[:, w:], qs[:, w:], rq[:], OP.mult)
                    # mm = -un = (x1000 - 1000) * x1000
                    nc.vector.scalar_tensor_tensor(
                        mm[:], xo[:], 1000.0, xo[:], OP.subtract, OP.mult
                    )
                    # am = 15625*i^2 + un
                    nc.vector.scalar_tensor_tensor(
                        am[:, :w], isq[:], 15625.0, mm[:, :w], OP.mult, OP.subtract
                    )
                    nc.vector.scalar_tensor_tensor(
                        am[:, w:], isq[:], 15625.0, mm[:, w:], OP.mult, OP.subtract
                    )
                    act(lam[:], am[:], AF.Ln, scale=400.0)

                    # ---- previous tile's MLP rides the exp/tanh table set ----
                    if prev is not None:
                        emit_mlp(*prev)
                    act(xo[:], lam[:], AF.Exp, scale=0.5)  # sqm (exp set too)

                    # nm = 2500*i + sqm  -> am
                    nc.vector.scalar_tensor_tensor(
                        am[:, :w], vin[:], 2500.0, xo[:, :w], OP.mult, OP.add
                    )
                    nc.vector.scalar_tensor_tensor(
                        am[:, w:], vin[:], 2500.0, xo[:, w:], OP.mult, OP.add
                    )
                    act(am[:], am[:], AF.Ln)               # lnum
                    act(mm[:], mm[:], AF.Ln, scale=-400.0)  # lun
                    # as2 = lnum - 0.5*lun -> xo
                    nc.vector.scalar_tensor_tensor(
                        xo[:], mm[:], -0.5, am[:], OP.mult, OP.add
                    )
                    nc.vector.scalar_tensor_tensor(
                        tnp[:, :w], xo[:, :w], float(C45), tbb[:], OP.mult, OP.mult
                    )
                    nc.vector.scalar_tensor_tensor(
                        tnp[:, w:], xo[:, w:], float(C45), tbb[:], OP.mult, OP.mult
                    )
                    nc.vector.tensor_tensor(se1[:], vin[:], rr[:], OP.mult)  # iro
                    nc.vector.scalar_tensor_tensor(
                        vo[:], vo[:], 0.9, se1[:], OP.mult, OP.add
                    )
                    nc.vector.scalar_tensor_tensor(
                        vsnp[:], vsnp[:], float(C89), tnp[:], OP.mult, OP.add
                    )
                    # a_n = cnB*qnB - i ; a_p = cnB*qpB + i  -> tnp halves
                    nc.vector.scalar_tensor_tensor(
                        tnp[:, :w], qb[:, :w], float(CNB), vin[:], OP.mult, OP.subtract
                    )
                    nc.vector.scalar_tensor_tensor(
                        tnp[:, w:], qb[:, w:], float(CNB), vin[:], OP.mult, OP.add
                    )
                    nc.vector.scalar_tensor_tensor(
                        qs[:], qs[:], float(C1S), tnp[:], OP.mult, OP.add
                    )
                    nc.vector.scalar_tensor_tensor(
                        qb[:], qb[:], float(C1B), tc2[:], OP.mult, OP.add
                    )
                    # x2 = qS2/qMax -> lam
                    nc.vector.tensor_tensor(lam[:, :w], qs[:, :w], rq[:], OP.mult)
                    nc.vector.tensor_tensor(lam[:, w:], qs[:, w:], rq[:], OP.mult)
                    act(xp16[:], lam[:, w:], AF.Copy)
                    act(mm[:], lam[:], AF.Ln, scale=-1.0, bias=1000.0)
                    act(am[:], lam[:], AF.Ln)
                    nc.vector.tensor_sub(se1[:], mm[:, w:], mm[:, :w])
                    nc.vector.tensor_sub(se2[:], am[:, w:], am[:, :w])
                    nc.vector.tensor_sub(se1[:], se1[:], se2[:])  # lnp - lnn
                    nc.vector.tensor_tensor(slt[:], tbb[:], se1[:], OP.mult)
                    # d3 = (Vo2 + negcb) + Vsn2 + Vsp2 ; s1 = (xn2*(-wn/1000) + lt) - d3
                    nc.vector.scalar_tensor_tensor(
                        sd2[:], vo[:], cbt[:], vsnp[:, :w], OP.add, OP.add
                    )
                    nc.vector.tensor_add(sd2[:], sd2[:], vsnp[:, w:])
                    nc.vector.scalar_tensor_tensor(
                        s1[:], lam[:, :w], wnt[:], slt[:], OP.mult, OP.add
                    )
                    nc.vector.tensor_sub(s1[:], s1[:], sd2[:])
                    # stores
                    nc.sync.dma_start(col_slice(1, s, xn_ap), vo[:])
                    nc.sync.dma_start(pair_slice(xn_ap, 2, 1, s), pair_tile(vsnp))
                    nc.sync.dma_start(pair_slice(xn_ap, 4, 2, s), pair_tile(qb))
                    nc.sync.dma_start(pair_slice(xn_ap, 5, 2, s), pair_tile(qs))

                    prev = (xp16, s1, s)

                emit_mlp(*prev)

    nc.compile()
    return nc


def _mlp_consts(Wp0, bp0, Wp2, bp2, Wp4, bp4, Wn, bn):
    l1 = np.zeros((128, 1024), np.float16)
    l2 = np.zeros((128, 256), np.float16)
    l3 = np.zeros((128, 512), np.float16)
    for ch in range(8):
        for g in range(16):
            for k in range(8):
                l1[ch * 16 + g, ch * 128 + g * 8 + k] = np.float16(Wp0[k, 0] / 1000.0)
    for e in range(2):
        for g in range(16):
            for k in range(8):
                for j in range(4):
                    l2[g * 8 + k, e * 128 + (e * 16 + g) * 4 + j] = np.float16(
                        Wp2[j, k]
                    )
    for pc in range(4):
        for gp in range(32):
            for j in range(4):
                l3[gp * 4 + j, pc * 128 + pc * 32 + gp] = np.float16(Wp4[0, j])
    b0 = np.tile(bp0.astype(np.float32), 16).reshape(128, 1)
    b2 = np.tile(bp2.astype(np.float32), 32).reshape(128, 1)
    negwn = np.full((128, 1), -Wn[0, 0] / 1000.0, np.float32)
    cbias = np.full((128, 1), -(U0P + bp4[0] - U0N - bn[0]), np.float32)
    return l1, l2, l3, b0, b2, negwn, cbias


_CACHE = {}

# test-harness hooks: set TRACE=True before calling kernel() to profile;
# the BassKernelResults of the last run lands in LAST_RESULTS.
TRACE = False
TRACE_KW = {}
LAST_RESULTS = None


def kernel(
    inputs, states, qMax, Ro, Wp0, bp0, Wp2, bp2, Wp4, bp4, Wn, bn, **unused
):
    inputs = np.ascontiguousarray(np.asarray(inputs, np.float32))
    states = np.asarray(states, np.float32)
    qMax = np.ascontiguousarray(np.asarray(qMax, np.float32))
    Ro = np.ascontiguousarray(np.asarray(Ro, np.float32))

    key = ("nc", NPC, W, T)
    if key not in _CACHE:
        _CACHE[key] = build_nc(NPC, W, T)
    nc = _CACHE[key]

    l1, l2, l3, b0, b2, negwn, cbias = _mlp_consts(
        np.asarray(Wp0, np.float32),
        np.asarray(bp0, np.float32),
        np.asarray(Wp2, np.float32),
        np.asarray(bp2, np.float32),
        np.asarray(Wp4, np.float32),
        np.asarray(bp4, np.float32),
        np.asarray(Wn, np.float32),
        np.asarray(bn, np.float32),
    )

    in_maps = []
    for k in range(NCORES):
        lo, hi = k * NPC, (k + 1) * NPC
        in_maps.append(
            {
                "inputs": inputs[lo:hi],
                "statesT": np.ascontiguousarray(states[lo:hi].T),
                "qMax": qMax[lo:hi],
                "Ro": Ro[lo:hi],
                "L1s": l1,
                "L2p": l2,
                "L3p": l3,
                "b0pat": b0,
                "b2pat": b2,
                "negwn": negwn,
                "cbias": cbias,
            }
        )

    res = run_bass_kernel_spmd(
        nc, in_maps, core_ids=list(range(NCORES)), trace=TRACE, **TRACE_KW
    )
    globals()["LAST_RESULTS"] = res
    V = np.concatenate([r["V"] for r in res.results], axis=0)
    XNew = np.concatenate(
        [np.ascontiguousarray(r["XNewT"].T) for r in res.results], axis=0
    )
    return V, XNew


# revision 25
# speedup vs baseline: 1.1115x; 1.1115x over previous
"""Trainium2 Bass kernel for nn_BatteryRNNCell — data-parallel over 8 NeuronCores.

Strategy
--------
Pure data parallel: batch (4M rows) split into 8 x 500K-row shards, one per
core. Each core processes its shard in T=4 tiles of [128 partitions x W=980]
rows (the last tile overlaps the previous one by 1760 rows; overlap rows are
recomputed with identical values, which avoids padding).

Layout: the host transposes `states` to column-major [8, N] before upload and
transposes XNew back after download, so every on-chip access is contiguous
(interleaved row-major state columns cost ~2x on DVE and ~10x on GPSIMD).
State columns are loaded as merged pair tiles (VSNP=[Vsn|Vsp], QB=[qnB|qpB],
QS=[qnS|qpS]) via 2-segment DMAs. Tb passes through via a direct DRAM->DRAM
copy.

Math restructuring (fp32 everywhere except the MLP matmuls):
  asinh(Jn/(2*Jn0)) = ln(2500*i + sqrt(6.25e6*i^2 + 400*un)) - 0.5*ln(400*un)
      with un = xn1000*(1000-xn1000), xn1000 = qnS/qMax  (no division by Jn0)
  sqrt(z) = exp(0.5*ln(z))   (keeps ScalarE inside the natural_log_exp table
      set; the kernel uses only two ACT table sets -> 2 table loads total)
  ln((1-x)/x) = ln(1000-x1000) - ln(x1000)
The 1->8->4->1 tanh MLP runs on the TensorEngine in fp16 (weights fp16,
activations fp16, PSUM accumulation fp32; ~2e-4 max abs error on the MLP
output) via block-replication matmuls; layer 2 packs two 16-row chunks per
PSUM tile so the tanh element count is 12/row instead of 16/row.

GPSIMD is not used at all: its strided ops measured ~19 cyc/elem and its
shared-SBUF-port lock stalls concurrent 2-source VectorE ops for the whole
GPSIMD instruction.
"""

import sys

if "/opt/trn_rl_repo" not in sys.path:
    sys.path.insert(0, "/opt/trn_rl_repo")

from contextlib import ExitStack

import numpy as np

import concourse.bass as bass  # noqa: F401
import concourse.mybir as mybir
import concourse.tile as tile
from concourse import bacc
from concourse.bass import _add_dep_helper
from concourse.bass_utils import run_bass_kernel_spmd

F32 = mybir.dt.float32
F16 = mybir.dt.float16
AF = mybir.ActivationFunctionType
OP = mybir.AluOpType

# ---- physics constants (from the reference model) ----
R_ = 8.3144621
F_ = 96487.0
VOL = 2.2e-05
VOLS = 0.1 * VOL
VOLB = VOL - VOLS
TD = 7.0e6
U0P = 4.03
U0N = 0.01

CB = np.float32(R_ / F_)                 # R/F
CNB = np.float32(1.0 / (VOLB * TD))
CNS = np.float32(1.0 / (VOLS * TD))
C1B = np.float32(1.0 - 1.0 / (VOLB * TD))
C1S = np.float32(1.0 - 1.0 / (VOLS * TD))
C89 = np.float32(1.0 - 1.0 / 90.0)
C45 = np.float32(2.0 / 90.0)             # (R/F/ALPHA)/TSN / (R/F)

NCORES = 8
P = 128
B_FULL = 4_000_000
NPC = B_FULL // NCORES  # 500_000

W = 980
T = 4


def _starts(npc, w, t):
    tile_rows = P * w
    s = [i * tile_rows for i in range(t - 1)]
    s.append(npc - tile_rows)
    assert s[-1] >= (s[-2] if t > 1 else 0)
    return s


def build_nc(npc=NPC, w=W, t_tiles=T):
    starts = _starts(npc, w, t_tiles)
    h = w // 2
    assert h <= 512

    nc = bacc.Bacc(
        "TRN2",
        target_bir_lowering=False,
        debug=False,
        enable_asserts=False,
    )
    # register the 1000.0 activation-bias constant (same pattern as Bass init)
    _k1000 = nc.alloc_sbuf_tensor("const-float32-1000.0", [128, 1], F32)
    nc.gpsimd.memset(_k1000.ap(), 1000.0)
    nc.const_aps.aps[(F32, 1000.0)] = _k1000.ap()
    nc.all_engine_barrier()

    d_inputs = nc.dram_tensor("inputs", [npc, 1], F32, kind="ExternalInput")
    d_states = nc.dram_tensor("statesT", [8, npc], F32, kind="ExternalInput")
    d_qmax = nc.dram_tensor("qMax", [npc], F32, kind="ExternalInput")
    d_ro = nc.dram_tensor("Ro", [npc], F32, kind="ExternalInput")
    d_l1 = nc.dram_tensor("L1s", [128, 1024], F16, kind="ExternalInput")
    d_l2 = nc.dram_tensor("L2p", [128, 256], F16, kind="ExternalInput")
    d_l3 = nc.dram_tensor("L3p", [128, 512], F16, kind="ExternalInput")
    d_b0 = nc.dram_tensor("b0pat", [128, 1], F32, kind="ExternalInput")
    d_b2 = nc.dram_tensor("b2pat", [128, 1], F32, kind="ExternalInput")
    d_wn = nc.dram_tensor("negwn", [128, 1], F32, kind="ExternalInput")
    d_cb = nc.dram_tensor("cbias", [128, 1], F32, kind="ExternalInput")

    d_v = nc.dram_tensor("V", [npc, 1], F32, kind="ExternalOutput")
    d_x = nc.dram_tensor("XNewT", [8, npc], F32, kind="ExternalOutput")

    st_ap = d_states.ap()
    xn_ap = d_x.ap()
    in_flat = d_inputs.ap().rearrange("r c -> (r c)")
    v_flat = d_v.ap().rearrange("r c -> (r c)")
    qm_ap = d_qmax.ap()
    ro_ap = d_ro.ap()

    def vec_slice(flat, s):
        return flat[s : s + P * w].rearrange("(p w) -> p w", w=w)

    def col_slice(c, s, ap=None):
        ap = st_ap if ap is None else ap
        return ap[c, s : s + P * w].rearrange("(p w) -> p w", w=w)

    def pair_slice(ap, c0, step, s):
        # [2, P*w] rows (c0, c0+step) -> 3-D AP matching a [P, 2w] tile
        # viewed as [P, 2, w] ([colA | colB] halves)
        return ap[c0 : c0 + step + 1 : step, s : s + P * w].rearrange(
            "c (p w) -> p c w", w=w
        )

    def pair_tile(t):
        return t[:].rearrange("p (c w) -> p c w", c=2)

    with tile.TileContext(nc) as tc:
        with ExitStack() as ctx:
            cpool = ctx.enter_context(tc.tile_pool(name="const", bufs=1))
            keep = ctx.enter_context(tc.tile_pool(name="keep", bufs=1))

            l1t = cpool.tile([128, 1024], F16, tag="l1")
            nc.sync.dma_start(l1t[:], d_l1.ap())
            l2t = cpool.tile([128, 256], F16, tag="l2")
            nc.sync.dma_start(l2t[:], d_l2.ap())
            l3t = cpool.tile([128, 512], F16, tag="l3")
            nc.sync.dma_start(l3t[:], d_l3.ap())
            b0t = cpool.tile([128, 1], F32, tag="b0")
            nc.sync.dma_start(b0t[:], d_b0.ap())
            b2t = cpool.tile([128, 1], F32, tag="b2")
            nc.sync.dma_start(b2t[:], d_b2.ap())
            wnt = cpool.tile([128, 1], F32, tag="wn")
            nc.sync.dma_start(wnt[:], d_wn.ap())
            cbt = cpool.tile([128, 1], F32, tag="cb")
            nc.sync.dma_start(cbt[:], d_cb.ap())

            with ExitStack() as actx:
                inp = actx.enter_context(tc.tile_pool(name="inp", bufs=2))
                big = actx.enter_context(tc.tile_pool(name="big", bufs=1))
                small = actx.enter_context(tc.tile_pool(name="small", bufs=1))
                keep = actx.enter_context(tc.tile_pool(name="keep", bufs=2))
                hpool = actx.enter_context(tc.tile_pool(name="hpool", bufs=1))
                opool = actx.enter_context(tc.tile_pool(name="opool", bufs=2))
                ppool = actx.enter_context(
                    tc.tile_pool(name="ppool", bufs=3, space="PSUM")
                )
                vpool = actx.enter_context(
                    tc.tile_pool(name="vpool", bufs=1, space="PSUM")
                )

                def _half(hh):
                    return slice(512 * hh, 512 * hh + h)

                last_act = [None]

                def act(*a, chain=True, **kw):
                    # serialize set-sensitive ScalarE ops in emission order so
                    # the scheduler cannot interleave table sets (each flip
                    # costs ~2.7us). Square/Copy live in every set -> leave
                    # them unchained to fill ACT idle gaps for free.
                    r = nc.scalar.activation(*a, **kw)
                    if chain:
                        if last_act[0] is not None:
                            _add_dep_helper(
                                r.ins, last_act[0].ins, sync=False,
                                reason="ACT order",
                            )
                        last_act[0] = r
                    return r

                def _half(hh):
                    return slice(512 * hh, 512 * hh + h)

                def emit_mlp(xp16, s1, s):
                    vml = vpool.tile([P, 1024], F32, tag="vml", name="vml")
                    h1s = []
                    for ch in range(8):
                        psa = ppool.tile([P, 1024], F32, tag="ps", name="psa")
                        for hh in range(2):
                            nc.tensor.matmul(
                                psa[:, _half(hh)],
                                l1t[:, ch * 128 : (ch + 1) * 128],
                                xp16[:, hh * h : (hh + 1) * h],
                                start=True,
                                stop=True,
                            )
                        h1 = hpool.tile(
                            [P, 1024], F16, tag=f"h1_{ch}", name=f"h1_{ch}"
                        )
                        act(h1[:], psa[:], AF.Tanh, bias=b0t[:])
                        h1s.append(h1)
                    h2s = []
                    for pc in range(4):
                        psb = ppool.tile([P, 1024], F32, tag="ps", name="psb")
                        for hh in range(2):
                            for e in range(2):
                                nc.tensor.matmul(
                                    psb[:, _half(hh)],
                                    l2t[:, e * 128 : (e + 1) * 128],
                                    h1s[2 * pc + e][:, _half(hh)],
                                    start=(e == 0),
                                    stop=(e == 1),
                                )
                        h2 = hpool.tile(
                            [P, 1024], F16, tag=f"h2_{pc}", name=f"h2_{pc}"
                        )
                        act(h2[:], psb[:], AF.Tanh, bias=b2t[:])
                        h2s.append(h2)
                    for hh in range(2):
                        for pc in range(4):
                            nc.tensor.matmul(
                                vml[:, _half(hh)],
                                l3t[:, pc * 128 : (pc + 1) * 128],
                                h2s[pc][:, _half(hh)],
                                start=(pc == 0),
                                stop=(pc == 3),
                            )
                    vout = opool.tile([P, w], F32, tag="vo2", name="vout", bufs=1)
                    vml3 = vml[:].rearrange("p (b k) -> p b k", k=512)[:, :, :h]
                    s13 = s1[:].rearrange("p (b k) -> p b k", k=h)
                    vo3 = vout[:].rearrange("p (b k) -> p b k", k=h)
                    nc.vector.scalar_tensor_tensor(
                        vo3, vml3, 0.0, s13, OP.add, OP.add
                    )
                    nc.sync.dma_start(vec_slice(v_flat, s), vout[:])

                prev = None  # (xp16, s1, start) of the previous tile
                for ti in range(t_tiles):
                    s = starts[ti]

                    xp16 = keep.tile([P, w], F16, tag="xp", name="xp16")
                    s1 = keep.tile([P, w], F32, tag="s1", name="s1")

                    vin = inp.tile([P, w], F32, tag="vin")
                    nc.sync.dma_start(vin[:], vec_slice(in_flat, s))
                    qq = inp.tile([P, w], F32, tag="qq", bufs=1)
                    nc.sync.dma_start(qq[:], vec_slice(qm_ap, s))
                    rr = inp.tile([P, w], F32, tag="rr", bufs=1)
                    nc.sync.dma_start(rr[:], vec_slice(ro_ap, s))
                    tb = inp.tile([P, w], F32, tag="tb", bufs=1)
                    nc.sync.dma_start(tb[:], col_slice(0, s))
                    vo = inp.tile([P, w], F32, tag="vo", bufs=1)
                    nc.sync.dma_start(vo[:], col_slice(1, s))
                    vsnp = inp.tile([P, 2 * w], F32, tag="vsnp")
                    nc.sync.dma_start(pair_tile(vsnp), pair_slice(st_ap, 2, 1, s))
                    qb = inp.tile([P, 2 * w], F32, tag="qb")
                    nc.sync.dma_start(pair_tile(qb), pair_slice(st_ap, 4, 2, s))
                    qs = inp.tile([P, 2 * w], F32, tag="qs")
                    nc.sync.dma_start(pair_tile(qs), pair_slice(st_ap, 5, 2, s))

                    # scratch tiles; several live multiple sequential lives
                    xo = big.tile([P, 2 * w], F32, tag="xo")    # x1000|sqm
                    mm = big.tile([P, 2 * w], F32, tag="mm", bufs=2)  # -un|lun
                    am = big.tile([P, 2 * w], F32, tag="am")    # a1|nm|lnum
                    lamt = big.tile([P, 2 * w], F32, tag="lamt")  # lam|lr2
                    x2t = big.tile([P, 2 * w], F32, tag="x2t")  # xn2|xp2
                    tnp = big.tile([P, 2 * w], F32, tag="tnp")  # an|ap , tn|tp
                    tc2 = big.tile([P, 2 * w], F32, tag="tc2")  # tc2|as2|lr1
                    rq = small.tile([P, w], F32, tag="rq")
                    isq = small.tile([P, w], F32, tag="isq")    # isq|iro|e1
                    tbb = small.tile([P, w], F32, tag="tbb")
                    se2 = small.tile([P, w], F32, tag="se2")    # e2|lt
                    sd2 = small.tile([P, w], F32, tag="sd2")

                    # --- early block: stoichiometry + diffusion (DVE) ---
                    nc.vector.reciprocal_approx_fast(rq[:], qq[:])
                    act(isq[:], vin[:], AF.Square, chain=False)
                    act(tbb[:], tb[:], AF.Copy, scale=float(CB), chain=False)
                    act(tc2[:], qs[:], AF.Copy, scale=float(CNS), chain=False)
                    nc.vector.tensor_tensor(xo[:, :w], qs[:, :w], rq[:], OP.mult)
                    nc.vector.tensor_tensor(xo[:, w:], qs[:, w:], rq[:], OP.mult)
                    nc.vector.scalar_tensor_tensor(
                        mm[:], xo[:], 1000.0, xo[:], OP.subtract, OP.mult
                    )
                    nc.vector.scalar_tensor_tensor(
                        tnp[:, :w], qb[:, :w], float(CNB), vin[:], OP.mult, OP.subtract
                    )
                    nc.vector.scalar_tensor_tensor(
                        tnp[:, w:], qb[:, w:], float(CNB), vin[:], OP.mult, OP.add
                    )
                    nc.vector.scalar_tensor_tensor(
                        qs[:], qs[:], float(C1S), tnp[:], OP.mult, OP.add
                    )
                    nc.vector.scalar_tensor_tensor(
                        qb[:], qb[:], float(C1B), tc2[:], OP.mult, OP.add
                    )
                    nc.sync.dma_start(pair_slice(xn_ap, 4, 2, s), pair_tile(qb))
                    nc.sync.dma_start(pair_slice(xn_ap, 5, 2, s), pair_tile(qs))
                    nc.vector.tensor_tensor(x2t[:, :w], qs[:, :w], rq[:], OP.mult)
                    nc.vector.tensor_tensor(x2t[:, w:], qs[:, w:], rq[:], OP.mult)
                    act(xp16[:], x2t[:, w:], AF.Copy, chain=False)
                    # am = 15625*i^2 + un
                    nc.vector.scalar_tensor_tensor(
                        am[:, :w], isq[:], 15625.0, mm[:, :w], OP.mult, OP.subtract
                    )
                    nc.vector.scalar_tensor_tensor(
                        am[:, w:], isq[:], 15625.0, mm[:, w:], OP.mult, OP.subtract
                    )
                    nc.vector.tensor_tensor(isq[:], vin[:], rr[:], OP.mult)  # iro
                    nc.vector.scalar_tensor_tensor(
                        vo[:], vo[:], 0.9, isq[:], OP.mult, OP.add
                    )
                    nc.sync.dma_start(col_slice(1, s, xn_ap), vo[:])
                    act(lamt[:], am[:], AF.Ln, scale=400.0)

                    # ---- previous tile's MLP rides the exp/tanh table set ----
                    if prev is not None:
                        emit_mlp(*prev)
                    act(xo[:], lamt[:], AF.Exp, scale=0.5)  # sqm

                    # nm = 2500*i + sqm  -> am
                    nc.vector.scalar_tensor_tensor(
                        am[:, :w], vin[:], 2500.0, xo[:, :w], OP.mult, OP.add
                    )
                    nc.vector.scalar_tensor_tensor(
                        am[:, w:], vin[:], 2500.0, xo[:, w:], OP.mult, OP.add
                    )
                    act(am[:], am[:], AF.Ln)                 # lnum
                    act(mm[:], mm[:], AF.Ln, scale=-400.0)   # lun
                    # as2 = lnum - 0.5*lun -> tc2
                    nc.vector.scalar_tensor_tensor(
                        tc2[:], mm[:], -0.5, am[:], OP.mult, OP.add
                    )
                    nc.vector.scalar_tensor_tensor(
                        tnp[:, :w], tc2[:, :w], float(C45), tbb[:], OP.mult, OP.mult
                    )
                    nc.vector.scalar_tensor_tensor(
                        tnp[:, w:], tc2[:, w:], float(C45), tbb[:], OP.mult, OP.mult
                    )
                    nc.vector.scalar_tensor_tensor(
                        vsnp[:], vsnp[:], float(C89), tnp[:], OP.mult, OP.add
                    )
                    nc.sync.dma_start(pair_slice(xn_ap, 2, 1, s), pair_tile(vsnp))
                    act(tc2[:], x2t[:], AF.Ln, scale=-1.0, bias=1000.0)  # lr1
                    act(lamt[:], x2t[:], AF.Ln)                          # lr2
                    # d3 = (Vo2 + negcb) + Vsn2 + Vsp2
                    nc.vector.scalar_tensor_tensor(
                        sd2[:], vo[:], cbt[:], vsnp[:, :w], OP.add, OP.add
                    )
                    nc.vector.tensor_add(sd2[:], sd2[:], vsnp[:, w:])
                    nc.vector.tensor_sub(isq[:], tc2[:, w:], tc2[:, :w])   # e1
                    nc.vector.tensor_sub(se2[:], lamt[:, w:], lamt[:, :w])  # e2
                    nc.vector.tensor_sub(isq[:], isq[:], se2[:])  # lnp - lnn
                    nc.vector.tensor_tensor(se2[:], tbb[:], isq[:], OP.mult)  # lt
                    nc.vector.scalar_tensor_tensor(
                        s1[:], x2t[:, :w], wnt[:], se2[:], OP.mult, OP.add
                    )
                    nc.vector.tensor_sub(s1[:], s1[:], sd2[:])

                    prev = (xp16, s1, s)

                emit_mlp(*prev)
                # Tb passes through unchanged: one DRAM->DRAM copy (emitted
                # last so it does not delay the first tile's input loads)
                nc.sync.dma_start(xn_ap[0:1, :], st_ap[0:1, :])

    nc.compile()
    return nc


def _mlp_consts(Wp0, bp0, Wp2, bp2, Wp4, bp4, Wn, bn):
    l1 = np.zeros((128, 1024), np.float16)
    l2 = np.zeros((128, 256), np.float16)
    l3 = np.zeros((128, 512), np.float16)
    for ch in range(8):
        for g in range(16):
            for k in range(8):
                l1[ch * 16 + g, ch * 128 + g * 8 + k] = np.float16(Wp0[k, 0] / 1000.0)
    for e in range(2):
        for g in range(16):
            for k in range(8):
                for j in range(4):
                    l2[g * 8 + k, e * 128 + (e * 16 + g) * 4 + j] = np.float16(
                        Wp2[j, k]
                    )
    for pc in range(4):
        for gp in range(32):
            for j in range(4):
                l3[gp * 4 + j, pc * 128 + pc * 32 + gp] = np.float16(Wp4[0, j])
    b0 = np.tile(bp0.astype(np.float32), 16).reshape(128, 1)
    b2 = np.tile(bp2.astype(np.float32), 32).reshape(128, 1)
    negwn = np.full((128, 1), -Wn[0, 0] / 1000.0, np.float32)
    cbias = np.full((128, 1), -(U0P + bp4[0] - U0N - bn[0]), np.float32)
    return l1, l2, l3, b0, b2, negwn, cbias


_CACHE = {}

# test-harness hooks: set TRACE=True before calling kernel() to profile;
# the BassKernelResults of the last run lands in LAST_RESULTS.
TRACE = False
TRACE_KW = {}
LAST_RESULTS = None


def kernel(
    inputs, states, qMax, Ro, Wp0, bp0, Wp2, bp2, Wp4, bp4, Wn, bn, **unused
):
    inputs = np.ascontiguousarray(np.asarray(inputs, np.float32))
    states = np.asarray(states, np.float32)
    qMax = np.ascontiguousarray(np.asarray(qMax, np.float32))
    Ro = np.ascontiguousarray(np.asarray(Ro, np.float32))

    key = ("nc", NPC, W, T)
    if key not in _CACHE:
        _CACHE[key] = build_nc(NPC, W, T)
    nc = _CACHE[key]

    l1, l2, l3, b0, b2, negwn, cbias = _mlp_consts(
        np.asarray(Wp0, np.float32),
        np.asarray(bp0, np.float32),
        np.asarray(Wp2, np.float32),
        np.asarray(bp2, np.float32),
        np.asarray(Wp4, np.float32),
        np.asarray(bp4, np.float32),
        np.asarray(Wn, np.float32),
        np.asarray(bn, np.float32),
    )

    in_maps = []
    for k in range(NCORES):
        lo, hi = k * NPC, (k + 1) * NPC
        in_maps.append(
            {
                "inputs": inputs[lo:hi],
                "statesT": np.ascontiguousarray(states[lo:hi].T),
                "qMax": qMax[lo:hi],
                "Ro": Ro[lo:hi],
                "L1s": l1,
                "L2p": l2,
                "L3p": l3,
                "b0pat": b0,
                "b2pat": b2,
                "negwn": negwn,
                "cbias": cbias,
            }
        )

    res = run_bass_kernel_spmd(
        nc, in_maps, core_ids=list(range(NCORES)), trace=TRACE, **TRACE_KW
    )
    globals()["LAST_RESULTS"] = res
    V = np.concatenate([r["V"] for r in res.results], axis=0)
    XNew = np.concatenate(
        [np.ascontiguousarray(r["XNewT"].T) for r in res.results], axis=0
    )
    return V, XNew


# revision 26
# speedup vs baseline: 1.1929x; 1.0732x over previous
"""Trainium2 Bass kernel for nn_BatteryRNNCell — data-parallel over 8 NeuronCores.

Strategy
--------
Pure data parallel: batch (4M rows) split into 8 x 500K-row shards, one per
core. Each core processes its shard in T=4 tiles of [128 partitions x W=980]
rows (the last tile overlaps the previous one by 1760 rows; overlap rows are
recomputed with identical values, which avoids padding).

Layout: the host transposes `states` to column-major [8, N] before upload and
transposes XNew back after download, so every on-chip access is contiguous
(interleaved row-major state columns cost ~2x on DVE and ~10x on GPSIMD).
State columns are loaded as merged pair tiles (VSNP=[Vsn|Vsp], QB=[qnB|qpB],
QS=[qnS|qpS]) via 2-segment DMAs. Tb passes through via a direct DRAM->DRAM
copy.

Math restructuring (fp32 everywhere except the MLP matmuls):
  asinh(Jn/(2*Jn0)) = ln(2500*i + sqrt(6.25e6*i^2 + 400*un)) - 0.5*ln(400*un)
      with un = xn1000*(1000-xn1000), xn1000 = qnS/qMax  (no division by Jn0)
  sqrt(z) = exp(0.5*ln(z))   (keeps ScalarE inside the natural_log_exp table
      set; the kernel uses only two ACT table sets -> 2 table loads total)
  ln((1-x)/x) = ln(1000-x1000) - ln(x1000)
The 1->8->4->1 tanh MLP runs on the TensorEngine in fp16 (weights fp16,
activations fp16, PSUM accumulation fp32; ~2e-4 max abs error on the MLP
output) via block-replication matmuls; layer 2 packs two 16-row chunks per
PSUM tile so the tanh element count is 12/row instead of 16/row.

GPSIMD is not used at all: its strided ops measured ~19 cyc/elem and its
shared-SBUF-port lock stalls concurrent 2-source VectorE ops for the whole
GPSIMD instruction.
"""

import sys

if "/opt/trn_rl_repo" not in sys.path:
    sys.path.insert(0, "/opt/trn_rl_repo")

from contextlib import ExitStack

import numpy as np

import concourse.bass as bass  # noqa: F401
import concourse.mybir as mybir
import concourse.tile as tile
from concourse import bacc
from concourse.bass import _add_dep_helper
from concourse.bass_utils import run_bass_kernel_spmd

F32 = mybir.dt.float32
F16 = mybir.dt.float16
AF = mybir.ActivationFunctionType
OP = mybir.AluOpType

# ---- physics constants (from the reference model) ----
R_ = 8.3144621
F_ = 96487.0
VOL = 2.2e-05
VOLS = 0.1 * VOL
VOLB = VOL - VOLS
TD = 7.0e6
U0P = 4.03
U0N = 0.01

CB = np.float32(R_ / F_)                 # R/F
CNB = np.float32(1.0 / (VOLB * TD))
CNS = np.float32(1.0 / (VOLS * TD))
C1B = np.float32(1.0 - 1.0 / (VOLB * TD))
C1S = np.float32(1.0 - 1.0 / (VOLS * TD))
C89 = np.float32(1.0 - 1.0 / 90.0)
C45 = np.float32(2.0 / 90.0)             # (R/F/ALPHA)/TSN / (R/F)

NCORES = 8
P = 128
B_FULL = 4_000_000
NPC = B_FULL // NCORES  # 500_000

W = 980
T = 4


def _starts(npc, w, t):
    tile_rows = P * w
    s = [i * tile_rows for i in range(t - 1)]
    s.append(npc - tile_rows)
    assert s[-1] >= (s[-2] if t > 1 else 0)
    return s


def build_nc(npc=NPC, w=W, t_tiles=T):
    starts = _starts(npc, w, t_tiles)
    h = w // 2
    assert h <= 512

    nc = bacc.Bacc(
        "TRN2",
        target_bir_lowering=False,
        debug=False,
        enable_asserts=False,
    )
    # register the 1000.0 activation-bias constant (same pattern as Bass init)
    _k1000 = nc.alloc_sbuf_tensor("const-float32-1000.0", [128, 1], F32)
    nc.gpsimd.memset(_k1000.ap(), 1000.0)
    nc.const_aps.aps[(F32, 1000.0)] = _k1000.ap()
    nc.all_engine_barrier()

    d_inputs = nc.dram_tensor("inputs", [npc, 1], F32, kind="ExternalInput")
    d_states = nc.dram_tensor("statesT", [8, npc], F32, kind="ExternalInput")
    d_qmax = nc.dram_tensor("qMax", [npc], F32, kind="ExternalInput")
    d_ro = nc.dram_tensor("Ro", [npc], F32, kind="ExternalInput")
    d_l1 = nc.dram_tensor("L1s", [128, 1024], F16, kind="ExternalInput")
    d_l2 = nc.dram_tensor("L2p", [128, 256], F16, kind="ExternalInput")
    d_l3 = nc.dram_tensor("L3p", [128, 512], F16, kind="ExternalInput")
    d_b0 = nc.dram_tensor("b0pat", [128, 1], F32, kind="ExternalInput")
    d_b2 = nc.dram_tensor("b2pat", [128, 1], F32, kind="ExternalInput")
    d_wn = nc.dram_tensor("negwn", [128, 1], F32, kind="ExternalInput")
    d_cb = nc.dram_tensor("cbias", [128, 1], F32, kind="ExternalInput")

    d_v = nc.dram_tensor("V", [npc, 1], F32, kind="ExternalOutput")
    d_x = nc.dram_tensor("XNewT", [8, npc], F32, kind="ExternalOutput")

    st_ap = d_states.ap()
    xn_ap = d_x.ap()
    in_flat = d_inputs.ap().rearrange("r c -> (r c)")
    v_flat = d_v.ap().rearrange("r c -> (r c)")
    qm_ap = d_qmax.ap()
    ro_ap = d_ro.ap()

    def vec_slice(flat, s):
        return flat[s : s + P * w].rearrange("(p w) -> p w", w=w)

    def col_slice(c, s, ap=None):
        ap = st_ap if ap is None else ap
        return ap[c, s : s + P * w].rearrange("(p w) -> p w", w=w)

    def pair_slice(ap, c0, step, s):
        # [2, P*w] rows (c0, c0+step) -> 3-D AP matching a [P, 2w] tile
        # viewed as [P, 2, w] ([colA | colB] halves)
        return ap[c0 : c0 + step + 1 : step, s : s + P * w].rearrange(
            "c (p w) -> p c w", w=w
        )

    def pair_tile(t):
        return t[:].rearrange("p (c w) -> p c w", c=2)

    with tile.TileContext(nc) as tc:
        with ExitStack() as ctx:
            cpool = ctx.enter_context(tc.tile_pool(name="const", bufs=1))
            keep = ctx.enter_context(tc.tile_pool(name="keep", bufs=1))

            l1t = cpool.tile([128, 1024], F16, tag="l1")
            nc.sync.dma_start(l1t[:], d_l1.ap())
            l2t = cpool.tile([128, 256], F16, tag="l2")
            nc.sync.dma_start(l2t[:], d_l2.ap())
            l3t = cpool.tile([128, 512], F16, tag="l3")
            nc.sync.dma_start(l3t[:], d_l3.ap())
            b0t = cpool.tile([128, 1], F32, tag="b0")
            nc.sync.dma_start(b0t[:], d_b0.ap())
            b2t = cpool.tile([128, 1], F32, tag="b2")
            nc.sync.dma_start(b2t[:], d_b2.ap())
            wnt = cpool.tile([128, 1], F32, tag="wn")
            nc.sync.dma_start(wnt[:], d_wn.ap())
            cbt = cpool.tile([128, 1], F32, tag="cb")
            nc.sync.dma_start(cbt[:], d_cb.ap())

            with ExitStack() as actx:
                inp = actx.enter_context(tc.tile_pool(name="inp", bufs=2))
                big = actx.enter_context(tc.tile_pool(name="big", bufs=1))
                small = actx.enter_context(tc.tile_pool(name="small", bufs=1))
                keep = actx.enter_context(tc.tile_pool(name="keep", bufs=2))
                hpool = actx.enter_context(tc.tile_pool(name="hpool", bufs=1))
                opool = actx.enter_context(tc.tile_pool(name="opool", bufs=2))
                ppool = actx.enter_context(
                    tc.tile_pool(name="ppool", bufs=3, space="PSUM")
                )
                vpool = actx.enter_context(
                    tc.tile_pool(name="vpool", bufs=1, space="PSUM")
                )

                def _half(hh):
                    return slice(512 * hh, 512 * hh + h)

                last_act = [None]

                def act(*a, chain=True, **kw):
                    # serialize set-sensitive ScalarE ops in emission order so
                    # the scheduler cannot interleave table sets (each flip
                    # costs ~2.7us). Square/Copy live in every set -> leave
                    # them unchained to fill ACT idle gaps for free.
                    r = nc.scalar.activation(*a, **kw)
                    if chain:
                        if last_act[0] is not None:
                            _add_dep_helper(
                                r.ins, last_act[0].ins, sync=False,
                                reason="ACT order",
                            )
                        last_act[0] = r
                    return r

                def _half(hh):
                    return slice(512 * hh, 512 * hh + h)

                def emit_mlp(xp16, s1, s):
                    vml = vpool.tile([P, 1024], F32, tag="vml", name="vml")
                    h1s = []
                    for ch in range(8):
                        psa = ppool.tile([P, 1024], F32, tag="ps", name="psa")
                        for hh in range(2):
                            nc.tensor.matmul(
                                psa[:, _half(hh)],
                                l1t[:, ch * 128 : (ch + 1) * 128],
                                xp16[:, hh * h : (hh + 1) * h],
                                start=True,
                                stop=True,
                            )
                        h1 = hpool.tile(
                            [P, 1024], F16, tag=f"h1_{ch}", name=f"h1_{ch}"
                        )
                        act(h1[:], psa[:], AF.Tanh, bias=b0t[:])
                        h1s.append(h1)
                    h2s = []
                    for pc in range(4):
                        psb = ppool.tile([P, 1024], F32, tag="ps", name="psb")
                        for hh in range(2):
                            for e in range(2):
                                nc.tensor.matmul(
                                    psb[:, _half(hh)],
                                    l2t[:, e * 128 : (e + 1) * 128],
                                    h1s[2 * pc + e][:, _half(hh)],
                                    start=(e == 0),
                                    stop=(e == 1),
                                )
                        h2 = hpool.tile(
                            [P, 1024], F16, tag=f"h2_{pc}", name=f"h2_{pc}"
                        )
                        act(h2[:], psb[:], AF.Tanh, bias=b2t[:])
                        h2s.append(h2)
                    for hh in range(2):
                        for pc in range(4):
                            nc.tensor.matmul(
                                vml[:, _half(hh)],
                                l3t[:, pc * 128 : (pc + 1) * 128],
                                h2s[pc][:, _half(hh)],
                                start=(pc == 0),
                                stop=(pc == 3),
                            )
                    vout = opool.tile([P, w], F32, tag="vo2", name="vout", bufs=1)
                    vml3 = vml[:].rearrange("p (b k) -> p b k", k=512)[:, :, :h]
                    s13 = s1[:].rearrange("p (b k) -> p b k", k=h)
                    vo3 = vout[:].rearrange("p (b k) -> p b k", k=h)
                    nc.vector.scalar_tensor_tensor(
                        vo3, vml3, 0.0, s13, OP.add, OP.add
                    )
                    nc.sync.dma_start(vec_slice(v_flat, s), vout[:])

                prev = None  # (xp16, s1, start) of the previous tile
                for ti in range(t_tiles):
                    s = starts[ti]

                    xp16 = keep.tile([P, w], F16, tag="xp", name="xp16")
                    s1 = keep.tile([P, w], F32, tag="s1", name="s1")

                    qq = inp.tile([P, w], F32, tag="qq", bufs=1)
                    nc.sync.dma_start(qq[:], vec_slice(qm_ap, s))
                    qs = inp.tile([P, 2 * w], F32, tag="qs")
                    nc.sync.dma_start(pair_tile(qs), pair_slice(st_ap, 5, 2, s))
                    vin = inp.tile([P, w], F32, tag="vin")
                    nc.sync.dma_start(vin[:], vec_slice(in_flat, s))
                    qb = inp.tile([P, 2 * w], F32, tag="qb")
                    nc.sync.dma_start(pair_tile(qb), pair_slice(st_ap, 4, 2, s))
                    rr = inp.tile([P, w], F32, tag="rr", bufs=1)
                    nc.sync.dma_start(rr[:], vec_slice(ro_ap, s))
                    tb = inp.tile([P, w], F32, tag="tb", bufs=1)
                    nc.sync.dma_start(tb[:], col_slice(0, s))
                    vo = inp.tile([P, w], F32, tag="vo", bufs=1)
                    nc.sync.dma_start(vo[:], col_slice(1, s))
                    vsnp = inp.tile([P, 2 * w], F32, tag="vsnp")
                    nc.sync.dma_start(pair_tile(vsnp), pair_slice(st_ap, 2, 1, s))

                    # scratch tiles; several live multiple sequential lives
                    xo = big.tile([P, 2 * w], F32, tag="xo")    # x1000 only
                    mm = big.tile([P, 2 * w], F32, tag="mm", bufs=2)  # -un|lun
                    am = big.tile([P, 2 * w], F32, tag="am")    # a1|nm|lnum
                    lamt = big.tile([P, 2 * w], F32, tag="lamt")  # lam|lr2
                    x2t = big.tile([P, 2 * w], F32, tag="x2t")  # xn2|xp2
                    tnp = big.tile([P, 2 * w], F32, tag="tnp", bufs=2)  # an|ap , tn|tp
                    tc2 = big.tile([P, 2 * w], F32, tag="tc2", bufs=2)  # tc2|as2|lr1
                    rq = small.tile([P, w], F32, tag="rq")
                    isq = small.tile([P, w], F32, tag="isq")    # isq|iro|e1
                    tbb = small.tile([P, w], F32, tag="tbb")
                    se2 = small.tile([P, w], F32, tag="se2")    # e2|lt
                    sd2 = small.tile([P, w], F32, tag="sd2")

                    # --- early block: stoichiometry + diffusion (DVE) ---
                    nc.vector.reciprocal_approx_fast(rq[:], qq[:])
                    act(isq[:], vin[:], AF.Square, chain=False)
                    act(tbb[:], tb[:], AF.Copy, scale=float(CB), chain=False)
                    act(tc2[:], qs[:], AF.Copy, scale=float(CNS), chain=False)
                    nc.vector.tensor_tensor(xo[:, :w], qs[:, :w], rq[:], OP.mult)
                    nc.vector.tensor_tensor(xo[:, w:], qs[:, w:], rq[:], OP.mult)
                    nc.vector.scalar_tensor_tensor(
                        mm[:], xo[:], 1000.0, xo[:], OP.subtract, OP.mult
                    )
                    nc.vector.scalar_tensor_tensor(
                        tnp[:, :w], qb[:, :w], float(CNB), vin[:], OP.mult, OP.subtract
                    )
                    nc.vector.scalar_tensor_tensor(
                        tnp[:, w:], qb[:, w:], float(CNB), vin[:], OP.mult, OP.add
                    )
                    nc.vector.scalar_tensor_tensor(
                        qs[:], qs[:], float(C1S), tnp[:], OP.mult, OP.add
                    )
                    nc.vector.scalar_tensor_tensor(
                        qb[:], qb[:], float(C1B), tc2[:], OP.mult, OP.add
                    )
                    nc.sync.dma_start(pair_slice(xn_ap, 4, 2, s), pair_tile(qb))
                    nc.sync.dma_start(pair_slice(xn_ap, 5, 2, s), pair_tile(qs))
                    nc.vector.tensor_tensor(x2t[:, :w], qs[:, :w], rq[:], OP.mult)
                    nc.vector.tensor_tensor(x2t[:, w:], qs[:, w:], rq[:], OP.mult)
                    act(xp16[:], x2t[:, w:], AF.Copy, chain=False)
                    # am = 15625*i^2 + un
                    nc.vector.scalar_tensor_tensor(
                        am[:, :w], isq[:], 15625.0, mm[:, :w], OP.mult, OP.subtract
                    )
                    nc.vector.scalar_tensor_tensor(
                        am[:, w:], isq[:], 15625.0, mm[:, w:], OP.mult, OP.subtract
                    )
                    nc.vector.tensor_tensor(isq[:], vin[:], rr[:], OP.mult)  # iro
                    nc.vector.scalar_tensor_tensor(
                        vo[:], vo[:], 0.9, isq[:], OP.mult, OP.add
                    )
                    nc.sync.dma_start(col_slice(1, s, xn_ap), vo[:])
                    act(lamt[:], am[:], AF.Ln, scale=400.0)

                    # ---- previous tile's MLP rides the exp/tanh table set ----
                    if prev is not None:
                        emit_mlp(*prev)
                    act(am[:], lamt[:], AF.Exp, scale=0.5)  # sqm (a1 dead)

                    # nm = 2500*i + sqm  -> am
                    nc.vector.scalar_tensor_tensor(
                        am[:, :w], vin[:], 2500.0, am[:, :w], OP.mult, OP.add
                    )
                    nc.vector.scalar_tensor_tensor(
                        am[:, w:], vin[:], 2500.0, am[:, w:], OP.mult, OP.add
                    )
                    act(am[:], am[:], AF.Ln)                 # lnum
                    act(mm[:], mm[:], AF.Ln, scale=-400.0)   # lun
                    # as2 = lnum - 0.5*lun -> tc2
                    nc.vector.scalar_tensor_tensor(
                        tc2[:], mm[:], -0.5, am[:], OP.mult, OP.add
                    )
                    nc.vector.scalar_tensor_tensor(
                        tnp[:, :w], tc2[:, :w], float(C45), tbb[:], OP.mult, OP.mult
                    )
                    nc.vector.scalar_tensor_tensor(
                        tnp[:, w:], tc2[:, w:], float(C45), tbb[:], OP.mult, OP.mult
                    )
                    nc.vector.scalar_tensor_tensor(
                        vsnp[:], vsnp[:], float(C89), tnp[:], OP.mult, OP.add
                    )
                    nc.sync.dma_start(pair_slice(xn_ap, 2, 1, s), pair_tile(vsnp))
                    act(tc2[:], x2t[:], AF.Ln, scale=-1.0, bias=1000.0)  # lr1
                    act(lamt[:], x2t[:], AF.Ln)                          # lr2
                    # d3 = (Vo2 + negcb) + Vsn2 + Vsp2
                    nc.vector.scalar_tensor_tensor(
                        sd2[:], vo[:], cbt[:], vsnp[:, :w], OP.add, OP.add
                    )
                    nc.vector.tensor_add(sd2[:], sd2[:], vsnp[:, w:])
                    nc.vector.tensor_sub(isq[:], tc2[:, w:], tc2[:, :w])   # e1
                    nc.vector.tensor_sub(se2[:], lamt[:, w:], lamt[:, :w])  # e2
                    nc.vector.tensor_sub(isq[:], isq[:], se2[:])  # lnp - lnn
                    nc.vector.tensor_tensor(se2[:], tbb[:], isq[:], OP.mult)  # lt
                    nc.vector.scalar_tensor_tensor(
                        s1[:], x2t[:, :w], wnt[:], se2[:], OP.mult, OP.add
                    )
                    nc.vector.tensor_sub(s1[:], s1[:], sd2[:])

                    prev = (xp16, s1, s)

                # Tb passes through unchanged: one DRAM->DRAM copy (not at
                # the start so it does not delay the first tile's loads, not
                # at the very end so it overlaps the final MLP)
                nc.sync.dma_start(xn_ap[0:1, :], st_ap[0:1, :])
                emit_mlp(*prev)

    nc.compile()
    return nc


def _mlp_consts(Wp0, bp0, Wp2, bp2, Wp4, bp4, Wn, bn):
    l1 = np.zeros((128, 1024), np.float16)
    l2 = np.zeros((128, 256), np.float16)
    l3 = np.zeros((128, 512), np.float16)
    for ch in range(8):
        for g in range(16):
            for k in range(8):
                l1[ch * 16 + g, ch * 128 + g * 8 + k] = np.float16(Wp0[k, 0] / 1000.0)
    for e in range(2):
        for g in range(16):
            for k in range(8):
                for j in range(4):
                    l2[g * 8 + k, e * 128 + (e * 16 + g) * 4 + j] = np.float16(
                        Wp2[j, k]
                    )
    for pc in range(4):
        for gp in range(32):
            for j in range(4):
                l3[gp * 4 + j, pc * 128 + pc * 32 + gp] = np.float16(Wp4[0, j])
    b0 = np.tile(bp0.astype(np.float32), 16).reshape(128, 1)
    b2 = np.tile(bp2.astype(np.float32), 32).reshape(128, 1)
    negwn = np.full((128, 1), -Wn[0, 0] / 1000.0, np.float32)
    cbias = np.full((128, 1), -(U0P + bp4[0] - U0N - bn[0]), np.float32)
    return l1, l2, l3, b0, b2, negwn, cbias


_CACHE = {}

# test-harness hooks: set TRACE=True before calling kernel() to profile;
# the BassKernelResults of the last run lands in LAST_RESULTS.
TRACE = False
TRACE_KW = {}
LAST_RESULTS = None


def kernel(
    inputs, states, qMax, Ro, Wp0, bp0, Wp2, bp2, Wp4, bp4, Wn, bn, **unused
):
    inputs = np.ascontiguousarray(np.asarray(inputs, np.float32))
    states = np.asarray(states, np.float32)
    qMax = np.ascontiguousarray(np.asarray(qMax, np.float32))
    Ro = np.ascontiguousarray(np.asarray(Ro, np.float32))

    key = ("nc", NPC, W, T)
    if key not in _CACHE:
        _CACHE[key] = build_nc(NPC, W, T)
    nc = _CACHE[key]

    l1, l2, l3, b0, b2, negwn, cbias = _mlp_consts(
        np.asarray(Wp0, np.float32),
        np.asarray(bp0, np.float32),
        np.asarray(Wp2, np.float32),
        np.asarray(bp2, np.float32),
        np.asarray(Wp4, np.float32),
        np.asarray(bp4, np.float32),
        np.asarray(Wn, np.float32),
        np.asarray(bn, np.float32),
    )

    in_maps = []
    for k in range(NCORES):
        lo, hi = k * NPC, (k + 1) * NPC
        in_maps.append(
            {
                "inputs": inputs[lo:hi],
                "statesT": np.ascontiguousarray(states[lo:hi].T),
                "qMax": qMax[lo:hi],
                "Ro": Ro[lo:hi],
                "L1s": l1,
                "L2p": l2,
                "L3p": l3,
                "b0pat": b0,
                "b2pat": b2,
                "negwn": negwn,
                "cbias": cbias,
            }
        )

    res = run_bass_kernel_spmd(
        nc, in_maps, core_ids=list(range(NCORES)), trace=TRACE, **TRACE_KW
    )
    globals()["LAST_RESULTS"] = res
    V = np.concatenate([r["V"] for r in res.results], axis=0)
    XNew = np.concatenate(
        [np.ascontiguousarray(r["XNewT"].T) for r in res.results], axis=0
    )
    return V, XNew
